# revision 25
# baseline (speedup 1.0000x reference)
"""Trainium2 Bass kernel for the GatedODEFlow problem.

Math: the reference iterates  a <- a + h*alpha(a) * (tgt - a)  where
alpha depends on a only through the low-rank projection (a - mu) @ U / S.
Since each step is a per-row convex blend toward the fixed vector tgt,
a_t = c_t * x + (1 - c_t) * tgt  for a per-row scalar c_t, and the
projection evolves affinely in c_t:

    proj_t = c_t * (x@W - tgt@W) + (tgt@W - mu@W)   with W = U / (S+1e-6)
    dist2_t = A * c_t^2 + B2 * c_t + C              (per-row A, B2; global C)
    alpha_t = exp(-dist2_t / (2*k*sigma^2))
    c_{t+1} = c_t * (1 - h * alpha_t),  c_0 = 1
    out = c_N * x + (1 - c_N) * tgt

The device computes q0 = x @ W (one matmul per row), the scalar
recurrence -> c, and z = c * x; the host adds the rank-1 term
(1-c) (x) tgt in fp32 during the (mandatory) upcast/unpack pass.

Layout: everything on-device runs in TRANSPOSED layout (d on
partitions, rows on the free axis), and the host packs x per core in
exact SBUF tile order (one contiguous [128 x JPG*R] slab per DMA), so
the projection contracts d directly with plain matmuls -- no on-chip
transposes -- and every hot DMA moves straight 8-16KB lines per
partition. I/O is bf16 both ways, halving HBM traffic vs fp32:
~32 MiB read + ~32 MiB written per core against the ~358 GB/s
per-core HBM roofline.

The macro schedule starts and ends with small (256-row) macroblocks so
the store stream starts early and the last-store tail is short; the
seven middle macroblocks are 512 rows.

Sharding: data-parallel across 8 cores along the batch dim; small
parameters replicated (per the problem's sharding hint).
"""

import math
import os
from contextlib import ExitStack

import numpy as np
import ml_dtypes

import concourse.bass as bass
import concourse.mybir as mybir
import concourse.tile as tile
from concourse import bacc
from concourse.masks import make_identity
from concourse.bass_utils import run_bass_kernel_spmd

F32 = mybir.dt.float32
BF16 = mybir.dt.bfloat16
AF = mybir.ActivationFunctionType
OP = mybir.AluOpType

N_CORES = 8
D = 4096
KSUB = 64
NJ = D // 128        # d-chunks (32)
NG = 2               # j-groups per macro (sub-DMA granularity)
JPG = NJ // NG       # j-chunks per group (16)


def _schedule(rows):
    """Macro sizes: small edges, 512-row body."""
    assert rows % 128 == 0
    if rows == 4096:
        return [256, 512, 512, 512, 512, 512, 512, 512, 256]
    # generic fallback: plain 512s (plus one remainder macro)
    sizes = []
    left = rows
    while left > 0:
        s = min(512, left)
        sizes.append(s)
        left -= s
    return sizes


_PROGRAM_CACHE: dict = {}
LAST_RESULT = None


def _build_program(rows: int, num_steps: int, neg_inv: float, exp_bias: float,
                   neg_h: float):
    sizes = _schedule(rows)
    nmacro = len(sizes)
    nsp = rows // 128     # total 128-row groups (c_all columns)
    total_elems = rows * D

    nc = bacc.Bacc("TRN2")
    # x/z are packed on the host in exact SBUF tile order (flat, since
    # macro sizes vary): block (m, g) is 128*JPG*Rm contiguous elements
    # laid out [128 partitions x (JPG * Rm)].
    xp_d = nc.dram_tensor("x", [total_elems], BF16, kind="ExternalInput")
    w_d = nc.dram_tensor("w", [D, KSUB], BF16, kind="ExternalInput")
    nqt_d = nc.dram_tensor("nqt", [KSUB, 1], F32, kind="ExternalInput")
    abr_d = nc.dram_tensor("abr", [128, 2], BF16, kind="ExternalInput")
    z_d = nc.dram_tensor("z", [total_elems], BF16, kind="ExternalOutput")
    c_d = nc.dram_tensor("c", [128, nsp], F32, kind="ExternalOutput")

    with ExitStack() as ctx:
        tc = ctx.enter_context(tile.TileContext(nc))
        singles = ctx.enter_context(tc.tile_pool(name="singles", bufs=1))
        xpool = ctx.enter_context(tc.tile_pool(name="xp", bufs=7))
        stkpool = ctx.enter_context(tc.tile_pool(name="stkp", bufs=2))
        crpool = ctx.enter_context(tc.tile_pool(name="crp", bufs=2))
        smpool = ctx.enter_context(tc.tile_pool(name="smp", bufs=2))
        pq = ctx.enter_context(tc.tile_pool(name="pq", bufs=3, space="PSUM"))
        pab = ctx.enter_context(tc.tile_pool(name="pab", bufs=2, space="PSUM"))
        pdt = ctx.enter_context(tc.tile_pool(name="pdt", bufs=1, space="PSUM"))
        pcr = ctx.enter_context(tc.tile_pool(name="pcr", bufs=2, space="PSUM"))

        identu = singles.tile([128, 128], BF16)
        make_identity(nc, identu)
        w_sb = singles.tile([128, NJ, KSUB], BF16)
        nc.scalar.dma_start(out=w_sb, in_=w_d[:, :].rearrange("(j p) k -> p j k", p=128))
        nqt_sb = singles.tile([KSUB, 1], F32)
        nc.scalar.dma_start(out=nqt_sb, in_=nqt_d[:, :])
        abr_sb = singles.tile([128, 2], BF16)
        nc.scalar.dma_start(out=abr_sb, in_=abr_d[:, :])
        ebias_sb = singles.tile([128, 1], F32)
        nc.vector.memset(ebias_sb, exp_bias)
        ones_row = singles.tile([1, 128], BF16)
        nc.vector.memset(ones_row, 1.0)
        c_all = singles.tile([128, nsp], F32)

        # DRAM offsets of each (macro, group) block and c columns
        blk_off = []
        sp_off = []
        off = 0
        spo = 0
        for m, Rm in enumerate(sizes):
            blk_off.append([off + g * 128 * JPG * Rm for g in range(NG)])
            off += 128 * NJ * Rm
            sp_off.append(spo)
            spo += Rm // 128

        def emit_group(st, g):
            """One sub-load + its projection matmuls."""
            m = st["m"]
            Rm = sizes[m]
            if "pinned" in st:
                xgt = st["pinned"][g]
            else:
                xgt = xpool.tile([128, JPG, 512], BF16, tag="xg", name="xgt")
            # contiguous head-of-tile view [128, JPG, Rm]
            xg = xgt[:, :, :].rearrange("p j r -> p (j r)")[
                :, 0 : JPG * Rm].rearrange("p (j r) -> p j r", j=JPG)
            nc.sync.dma_start(
                out=xg,
                in_=xp_d[blk_off[m][g] : blk_off[m][g] + 128 * JPG * Rm]
                .rearrange("(p j r) -> p j r", p=128, j=JPG))
            st["xgs"].append(xg)
            for j in range(JPG):
                jj = g * JPG + j
                nc.tensor.matmul(
                    st["q0"], w_sb[:, jj, :], xg[:, j, :],
                    start=(jj == 0), stop=(jj == NJ - 1))

        def emit_stk_ab(st):
            """stk + A/B reduction after the full projection."""
            m = st["m"]
            Rm = sizes[m]
            spm = Rm // 128
            q0 = st["q0"]
            # stk rows 0..63 = (q0 - qT)^2 ; rows 64..127 = (q0 - qT)
            stk = stkpool.tile([128, 512], BF16, tag="stk", name="stk")[:, 0:Rm]
            nc.scalar.activation(stk[0:KSUB, :], q0, AF.Square,
                                 bias=nqt_sb, scale=1.0)
            nc.scalar.activation(stk[KSUB:128, :], q0, AF.Identity,
                                 bias=nqt_sb, scale=1.0)
            ab = pab.tile([128, 8], F32, tag="ab", name="ab")
            for s in range(spm):
                lhs = stk[:, s * 128 : (s + 1) * 128]
                nc.tensor.matmul(ab[:, s : s + 1], lhs,
                                 abr_sb[:, 0:1], start=True, stop=True)
                nc.tensor.matmul(ab[:, 4 + s : 4 + s + 1], lhs,
                                 abr_sb[:, 1:2], start=True, stop=True)
            st["ab"] = ab

        def emit_rec(st):
            """Per-row scalar recurrence (DVE + ACT exp) -> c (fp32)."""
            m = st["m"]
            spm = sizes[m] // 128
            ab = st["ab"]
            A = ab[:, 0:spm]
            B2 = ab[:, 4 : 4 + spm]
            c = smpool.tile([128, 4], F32, tag="c", name="c")[:, 0:spm]
            nc.vector.memset(c, 1.0)
            t1 = smpool.tile([128, 4], F32, tag="t1", name="t1")[:, 0:spm]
            alpha = smpool.tile([128, 4], F32, tag="alpha", name="alpha")[:, 0:spm]
            for _t in range(num_steps):
                nc.vector.tensor_tensor(t1, A, c, OP.mult)
                nc.vector.tensor_tensor(t1, t1, B2, OP.add)
                nc.vector.tensor_tensor(t1, t1, c, OP.mult)
                nc.scalar.activation(alpha, t1, AF.Exp,
                                     bias=ebias_sb, scale=neg_inv)
                nc.vector.tensor_tensor(t1, alpha, c, OP.mult)
                nc.vector.scalar_tensor_tensor(c, t1, neg_h, c, OP.mult, OP.add)
            o = sp_off[m]
            nc.vector.tensor_copy(c_all[:, o : o + spm], c)
            c16 = smpool.tile([128, 4], BF16, tag="c16", name="c16")[:, 0:spm]
            nc.vector.tensor_copy(c16, c)
            st["c16"] = c16

        def emit_crep(st):
            """c (per 128-row groups) -> crep [128, Rm] bf16 (PE + ACT)."""
            m = st["m"]
            Rm = sizes[m]
            spm = Rm // 128
            c16 = st["c16"]
            dT = pdt.tile([1, 512], BF16, tag="dT", name="dT")[:, 0:Rm]
            for s in range(spm):
                nc.tensor.transpose(dT[:, s * 128 : (s + 1) * 128],
                                    c16[:, s : s + 1], identu)
            crow = smpool.tile([1, 512], BF16, tag="crow", name="crow")[:, 0:Rm]
            nc.vector.tensor_copy(crow, dT)
            cps = pcr.tile([128, 512], F32, tag="cps", name="cps")[:, 0:Rm]
            nc.tensor.matmul(cps, ones_row, crow, start=True, stop=True)
            crep = crpool.tile([128, 512], BF16, tag="crep", name="crep")[:, 0:Rm]
            nc.scalar.copy(crep, cps)
            st["crep"] = crep

        def emit_back(st, defer_store=False):
            """z = c * x in place (DVE for g0, GPSIMD for g1), then store."""
            m, crep = st["m"], st["crep"]
            Rm = sizes[m]
            c3 = crep[:, :].rearrange("p (one r) -> p one r", one=1)
            for g in range(NG):
                xg = st["xgs"][g]
                _, bc = bass.broadcast_tensor_aps(xg[:, :, :], c3)
                eng = nc.vector if g == 0 else nc.gpsimd
                eng.tensor_tensor(xg, xg, bc, OP.mult)
                if not defer_store:
                    nc.scalar.dma_start(out=z_out_ap(m, g), in_=xg)

        def z_out_ap(m, g):
            Rm = sizes[m]
            return z_d[blk_off[m][g] : blk_off[m][g] + 128 * JPG * Rm] \
                .rearrange("(p j r) -> p j r", p=128, j=JPG)

        # Software-pipelined emission: macro m's blend/stores are emitted
        # after macro m+1's load+projection so no engine head-of-line
        # blocks on the serial gate recurrence. Macro E's z is computed
        # mid-stream into pinned tiles but its stores are deferred to the
        # very end ON THE LOAD RING: they fire the instant the last load
        # completes, hiding the final macro's recurrence chain under pure
        # DMA drain.
        E_SET = [2, 3] if nmacro >= 6 else []
        xgE = {e: [singles.tile([128, JPG, 512], BF16, name=f"xgE{e}_{g}")
                   for g in range(NG)] for e in E_SET}

        prev = None
        for m in range(nmacro):
            st = {"m": m, "xgs": [],
                  "q0": pq.tile([KSUB, 512], F32, tag="q0",
                                name="q0")[:, 0 : sizes[m]]}
            if m in xgE:
                st["pinned"] = xgE[m]
            emit_group(st, 0)
            emit_group(st, 1)
            emit_stk_ab(st)
            if prev is not None:
                emit_back(prev, defer_store=(prev["m"] in xgE))
            emit_rec(st)
            emit_crep(st)
            if m == nmacro - 1:
                # c output: only needs the recurrences; overlaps the tail
                nc.scalar.dma_start(out=c_d[:, :], in_=c_all)
            prev = st
        emit_back(prev, defer_store=(prev["m"] in xgE))
        # Deferred stores ride the (now-idle) load ring: they fire the
        # moment the last load completes, covering the final macros'
        # recurrence/blend chains with pure DMA drain.
        for e in E_SET:
            for g in range(NG):
                nc.sync.dma_start(out=z_out_ap(e, g), in_=xgE[e][g])

    if not nc.is_finalized():
        nc.finalize()
    return nc


def _get_program(rows, num_steps, neg_inv, exp_bias, neg_h):
    key = (rows, num_steps, neg_inv, exp_bias, neg_h)
    if key not in _PROGRAM_CACHE:
        _PROGRAM_CACHE[key] = _build_program(rows, num_steps, neg_inv,
                                             exp_bias, neg_h)
    return _PROGRAM_CACHE[key]


def _bf16_rtn(x32: np.ndarray) -> np.ndarray:
    """fp32 -> bf16 with round-to-nearest-even (no inf/nan handling)."""
    u = x32.view(np.uint32)
    r = (u + np.uint32(0x7FFF) + ((u >> np.uint32(16)) & np.uint32(1)))
    return (r >> np.uint32(16)).astype(np.uint16).view(ml_dtypes.bfloat16)


def _pack_core(xb_core: np.ndarray, sizes) -> np.ndarray:
    """[rows, D] bf16 -> flat packed (m, g, p, j, r) order."""
    parts = []
    r0 = 0
    for Rm in sizes:
        xv = xb_core[r0 : r0 + Rm].reshape(Rm, NG, JPG, 128)
        parts.append(np.ascontiguousarray(xv.transpose(1, 3, 2, 0)).reshape(-1))
        r0 += Rm
    return np.concatenate(parts)


def kernel(x, manifold_mu, manifold_U, manifold_S, attractor_mu,
           log_step, sigma, num_steps):
    global LAST_RESULT
    x = np.ascontiguousarray(np.asarray(x, dtype=np.float32))
    mu = np.asarray(manifold_mu, dtype=np.float64)
    U = np.asarray(manifold_U, dtype=np.float64)
    S = np.asarray(manifold_S, dtype=np.float64)
    tgt = np.asarray(attractor_mu, dtype=np.float64)
    ls = float(np.asarray(log_step))
    sg = float(np.asarray(sigma))
    ns = int(np.asarray(num_steps))

    batch, dmodel = x.shape
    assert dmodel == D and batch % N_CORES == 0

    if ns <= 0:
        return x.copy()

    # Host-side parameter folding (O(D*K), trivial). qT/wt/C use the
    # bf16-rounded W so they are consistent with the device projection.
    W = U / (S + 1e-6)[None, :]
    W16 = W.astype(ml_dtypes.bfloat16)
    Wq = W16.astype(np.float64)
    qT = tgt @ Wq
    qmu = mu @ Wq
    wt = qT - qmu
    Cc = float(wt @ wt)
    inv = 1.0 / (float(KSUB) * 2.0 * sg * sg * 1.0)  # TEMPERATURE = 1.0
    step = min(max(math.exp(ls), 1e-3), 1.0)
    h = step / ns

    neg_inv = -inv
    exp_bias = -inv * Cc
    neg_h = -h

    rows = batch // N_CORES
    nc = _get_program(rows, ns, neg_inv, exp_bias, neg_h)
    sizes = _schedule(rows)

    abr = np.zeros((128, 2), ml_dtypes.bfloat16)
    abr[0:KSUB, 0] = 1.0
    abr[KSUB:128, 1] = (2.0 * wt).astype(ml_dtypes.bfloat16)
    common = {
        "w": np.ascontiguousarray(W16),
        "nqt": np.ascontiguousarray((-qT).astype(np.float32)[:, None]),
        "abr": abr,
    }

    xb = _bf16_rtn(x)
    in_maps = []
    for i in range(N_CORES):
        xpk = _pack_core(xb[i * rows : (i + 1) * rows], sizes)
        in_maps.append({"x": xpk, **common})

    trace = bool(int(os.environ.get("GOF_TRACE", "0")))
    res = run_bass_kernel_spmd(nc, in_maps, list(range(N_CORES)), trace=trace)
    LAST_RESULT = res

    t32 = tgt.astype(np.float32)
    out = np.empty((batch, D), np.float32)
    for i in range(N_CORES):
        cc = np.asarray(res.results[i]["c"], np.float32)
        c_core = np.ascontiguousarray(cc.T).reshape(-1)
        zu = np.asarray(res.results[i]["z"]).view(np.uint16)
        oc = out[i * rows : (i + 1) * rows]
        r0 = 0
        off = 0
        for Rm in sizes:
            n = 128 * NJ * Rm
            zv = zu[off : off + n].reshape(NG, 128, JPG, Rm) \
                .transpose(3, 0, 2, 1)  # (r, g, j, p)
            np.left_shift(zv.astype(np.uint32).reshape(Rm, D), 16,
                          out=oc[r0 : r0 + Rm].view(np.uint32))
            r0 += Rm
            off += n
        oc += np.outer(1.0 - c_core, t32)
    return out


# revision 27
# speedup vs baseline: 1.3531x; 1.3531x over previous
"""Trainium2 Bass kernel for the GatedODEFlow problem.

Math: the reference iterates  a <- a + h*alpha(a) * (tgt - a)  where
alpha depends on a only through the low-rank projection (a - mu) @ U / S.
Since each step is a per-row convex blend toward the fixed vector tgt,
a_t = c_t * x + (1 - c_t) * tgt  for a per-row scalar c_t, and the
projection evolves affinely in c_t:

    proj_t = c_t * (x@W - tgt@W) + (tgt@W - mu@W)   with W = U / (S+1e-6)
    dist2_t = A * c_t^2 + B2 * c_t + C              (per-row A, B2; global C)
    alpha_t = exp(-dist2_t / (2*k*sigma^2))
    c_{t+1} = c_t * (1 - h * alpha_t),  c_0 = 1
    out = c_N * x + (1 - c_N) * tgt

The device computes q0 = x @ W (one matmul per row), the scalar
recurrence -> c, and z = c * x; the host adds the rank-1 term
(1-c) (x) tgt in fp32 during the (mandatory) upcast/unpack pass.

Layout: everything on-device runs in TRANSPOSED layout (d on
partitions, rows on the free axis), and the host packs x per core in
exact SBUF tile order (one contiguous [128 x JPG*R] slab per DMA), so
the projection contracts d directly with plain matmuls -- no on-chip
transposes -- and every hot DMA moves straight 8-16KB lines per
partition. I/O is bf16 both ways, halving HBM traffic vs fp32:
~32 MiB read + ~32 MiB written per core against the ~358 GB/s
per-core HBM roofline.

The macro schedule starts and ends with small (256-row) macroblocks so
the store stream starts early and the last-store tail is short; the
seven middle macroblocks are 512 rows.

Sharding: data-parallel across 8 cores along the batch dim; small
parameters replicated (per the problem's sharding hint).
"""

import math
import os
from contextlib import ExitStack

import numpy as np
import ml_dtypes

import concourse.bass as bass
import concourse.mybir as mybir
import concourse.tile as tile
from concourse import bacc
from concourse.masks import make_identity
from concourse.bass_utils import run_bass_kernel_spmd

F32 = mybir.dt.float32
BF16 = mybir.dt.bfloat16
AF = mybir.ActivationFunctionType
OP = mybir.AluOpType

N_CORES = 8
D = 4096
KSUB = 64
NJ = D // 128        # d-chunks (32)
NG = 2               # j-groups per macro (sub-DMA granularity)
JPG = NJ // NG       # j-chunks per group (16)


def _schedule(rows):
    """Macro sizes: small edges, 512-row body."""
    assert rows % 128 == 0
    if rows == 4096:
        return [256, 512, 512, 512, 512, 512, 512, 512, 256]
    # generic fallback: plain 512s (plus one remainder macro)
    sizes = []
    left = rows
    while left > 0:
        s = min(512, left)
        sizes.append(s)
        left -= s
    return sizes


_PROGRAM_CACHE: dict = {}
LAST_RESULT = None


def _build_program(rows: int, num_steps: int, neg_inv: float, exp_bias: float,
                   neg_h: float):
    sizes = _schedule(rows)
    nmacro = len(sizes)
    nsp = rows // 128     # total 128-row groups (c_all columns)
    total_elems = rows * D

    nc = bacc.Bacc("TRN2")
    # x/z are packed on the host in exact SBUF tile order (flat, since
    # macro sizes vary): block (m, g) is 128*JPG*Rm contiguous elements
    # laid out [128 partitions x (JPG * Rm)].
    xp_d = nc.dram_tensor("x", [total_elems], BF16, kind="ExternalInput")
    w_d = nc.dram_tensor("w", [D, KSUB], BF16, kind="ExternalInput")
    nqt_d = nc.dram_tensor("nqt", [KSUB, 1], F32, kind="ExternalInput")
    abr_d = nc.dram_tensor("abr", [128, 2], BF16, kind="ExternalInput")
    z_d = nc.dram_tensor("z", [total_elems], BF16, kind="ExternalOutput")
    c_d = nc.dram_tensor("c", [128, nsp], F32, kind="ExternalOutput")

    with ExitStack() as ctx:
        tc = ctx.enter_context(tile.TileContext(nc))
        singles = ctx.enter_context(tc.tile_pool(name="singles", bufs=1))
        xpool = ctx.enter_context(tc.tile_pool(name="xp", bufs=7))
        stkpool = ctx.enter_context(tc.tile_pool(name="stkp", bufs=2))
        crpool = ctx.enter_context(tc.tile_pool(name="crp", bufs=2))
        smpool = ctx.enter_context(tc.tile_pool(name="smp", bufs=2))
        pq = ctx.enter_context(tc.tile_pool(name="pq", bufs=3, space="PSUM"))
        pab = ctx.enter_context(tc.tile_pool(name="pab", bufs=2, space="PSUM"))
        pdt = ctx.enter_context(tc.tile_pool(name="pdt", bufs=1, space="PSUM"))
        pcr = ctx.enter_context(tc.tile_pool(name="pcr", bufs=2, space="PSUM"))

        identu = singles.tile([128, 128], BF16)
        make_identity(nc, identu)
        w_sb = singles.tile([128, NJ, KSUB], BF16)
        nc.scalar.dma_start(out=w_sb, in_=w_d[:, :].rearrange("(j p) k -> p j k", p=128))
        nqt_sb = singles.tile([KSUB, 1], F32)
        nc.scalar.dma_start(out=nqt_sb, in_=nqt_d[:, :])
        abr_sb = singles.tile([128, 2], BF16)
        nc.scalar.dma_start(out=abr_sb, in_=abr_d[:, :])
        ebias_sb = singles.tile([128, 1], F32)
        nc.vector.memset(ebias_sb, exp_bias)
        ones_row = singles.tile([1, 128], BF16)
        nc.vector.memset(ones_row, 1.0)
        c_all = singles.tile([128, nsp], F32)

        # DRAM offsets of each (macro, group) block and c columns
        blk_off = []
        sp_off = []
        off = 0
        spo = 0
        for m, Rm in enumerate(sizes):
            blk_off.append([off + g * 128 * JPG * Rm for g in range(NG)])
            off += 128 * NJ * Rm
            sp_off.append(spo)
            spo += Rm // 128

        def emit_group(st, g):
            """One sub-load + its projection matmuls."""
            m = st["m"]
            Rm = sizes[m]
            if "pinned" in st:
                xgt = st["pinned"][g]
            else:
                xgt = xpool.tile([128, JPG, 512], BF16, tag="xg", name="xgt")
            # contiguous head-of-tile view [128, JPG, Rm]
            xg = xgt[:, :, :].rearrange("p j r -> p (j r)")[
                :, 0 : JPG * Rm].rearrange("p (j r) -> p j r", j=JPG)
            nc.sync.dma_start(
                out=xg,
                in_=xp_d[blk_off[m][g] : blk_off[m][g] + 128 * JPG * Rm]
                .rearrange("(p j r) -> p j r", p=128, j=JPG))
            st["xgs"].append(xg)
            for j in range(JPG):
                jj = g * JPG + j
                nc.tensor.matmul(
                    st["q0"], w_sb[:, jj, :], xg[:, j, :],
                    start=(jj == 0), stop=(jj == NJ - 1))

        def emit_stk_ab(st):
            """stk + A/B reduction after the full projection."""
            m = st["m"]
            Rm = sizes[m]
            spm = Rm // 128
            q0 = st["q0"]
            # stk rows 0..63 = (q0 - qT)^2 ; rows 64..127 = (q0 - qT)
            stk = stkpool.tile([128, 512], BF16, tag="stk", name="stk")[:, 0:Rm]
            nc.scalar.activation(stk[0:KSUB, :], q0, AF.Square,
                                 bias=nqt_sb, scale=1.0)
            nc.scalar.activation(stk[KSUB:128, :], q0, AF.Identity,
                                 bias=nqt_sb, scale=1.0)
            ab = pab.tile([128, 8], F32, tag="ab", name="ab")
            for s in range(spm):
                lhs = stk[:, s * 128 : (s + 1) * 128]
                nc.tensor.matmul(ab[:, s : s + 1], lhs,
                                 abr_sb[:, 0:1], start=True, stop=True)
                nc.tensor.matmul(ab[:, 4 + s : 4 + s + 1], lhs,
                                 abr_sb[:, 1:2], start=True, stop=True)
            st["ab"] = ab

        def emit_rec(st):
            """Per-row scalar recurrence (DVE + ACT exp) -> c (fp32)."""
            m = st["m"]
            spm = sizes[m] // 128
            ab = st["ab"]
            A = ab[:, 0:spm]
            B2 = ab[:, 4 : 4 + spm]
            c = smpool.tile([128, 4], F32, tag="c", name="c")[:, 0:spm]
            nc.vector.memset(c, 1.0)
            t1 = smpool.tile([128, 4], F32, tag="t1", name="t1")[:, 0:spm]
            alpha = smpool.tile([128, 4], F32, tag="alpha", name="alpha")[:, 0:spm]
            for _t in range(num_steps):
                nc.vector.tensor_tensor(t1, A, c, OP.mult)
                nc.vector.tensor_tensor(t1, t1, B2, OP.add)
                nc.vector.tensor_tensor(t1, t1, c, OP.mult)
                nc.scalar.activation(alpha, t1, AF.Exp,
                                     bias=ebias_sb, scale=neg_inv)
                nc.vector.tensor_tensor(t1, alpha, c, OP.mult)
                nc.vector.scalar_tensor_tensor(c, t1, neg_h, c, OP.mult, OP.add)
            o = sp_off[m]
            nc.vector.tensor_copy(c_all[:, o : o + spm], c)
            c16 = smpool.tile([128, 4], BF16, tag="c16", name="c16")[:, 0:spm]
            nc.vector.tensor_copy(c16, c)
            st["c16"] = c16

        def emit_crep(st):
            """c (per 128-row groups) -> crep [128, Rm] bf16 (PE + ACT)."""
            m = st["m"]
            Rm = sizes[m]
            spm = Rm // 128
            c16 = st["c16"]
            dT = pdt.tile([1, 512], BF16, tag="dT", name="dT")[:, 0:Rm]
            for s in range(spm):
                nc.tensor.transpose(dT[:, s * 128 : (s + 1) * 128],
                                    c16[:, s : s + 1], identu)
            crow = smpool.tile([1, 512], BF16, tag="crow", name="crow")[:, 0:Rm]
            nc.vector.tensor_copy(crow, dT)
            cps = pcr.tile([128, 512], F32, tag="cps", name="cps")[:, 0:Rm]
            nc.tensor.matmul(cps, ones_row, crow, start=True, stop=True)
            crep = crpool.tile([128, 512], BF16, tag="crep", name="crep")[:, 0:Rm]
            nc.scalar.copy(crep, cps)
            st["crep"] = crep

        def emit_back(st, defer_store=False):
            """z = c * x in place (DVE for g0, GPSIMD for g1), then store."""
            m, crep = st["m"], st["crep"]
            Rm = sizes[m]
            c3 = crep[:, :].rearrange("p (one r) -> p one r", one=1)
            for g in range(NG):
                xg = st["xgs"][g]
                _, bc = bass.broadcast_tensor_aps(xg[:, :, :], c3)
                nc.vector.tensor_tensor(xg, xg, bc, OP.mult)
                if not defer_store:
                    nc.scalar.dma_start(out=z_out_ap(m, g), in_=xg)

        def z_out_ap(m, g):
            Rm = sizes[m]
            return z_d[blk_off[m][g] : blk_off[m][g] + 128 * JPG * Rm] \
                .rearrange("(p j r) -> p j r", p=128, j=JPG)

        # Software-pipelined emission: macro m's blend/stores are emitted
        # after macro m+1's load+projection so no engine head-of-line
        # blocks on the serial gate recurrence. Macro E's z is computed
        # mid-stream into pinned tiles but its stores are deferred to the
        # very end ON THE LOAD RING: they fire the instant the last load
        # completes, hiding the final macro's recurrence chain under pure
        # DMA drain.
        E_SET = [2, 3] if nmacro >= 6 else []
        xgE = {e: [singles.tile([128, JPG, 512], BF16, name=f"xgE{e}_{g}")
                   for g in range(NG)] for e in E_SET}

        # Two-deep software pipeline: macro m's crep (PE transposes +
        # outer product, which depend on the serial recurrence) and its
        # blend/stores are emitted one full iteration later, so the
        # recurrence of m always completes under macro m+1's projection
        # matmuls and the in-order PE stream never stalls on it.
        prev = None
        for m in range(nmacro):
            st = {"m": m, "xgs": [],
                  "q0": pq.tile([KSUB, 512], F32, tag="q0",
                                name="q0")[:, 0 : sizes[m]]}
            if m in xgE:
                st["pinned"] = xgE[m]
            emit_group(st, 0)
            emit_group(st, 1)
            emit_stk_ab(st)
            if prev is not None:
                emit_crep(prev)
                emit_back(prev, defer_store=(prev["m"] in xgE))
            emit_rec(st)
            if m == nmacro - 1:
                # c output: only needs the recurrences; overlaps the tail
                nc.scalar.dma_start(out=c_d[:, :], in_=c_all)
            prev = st
        emit_crep(prev)
        emit_back(prev, defer_store=(prev["m"] in xgE))
        # Deferred stores ride the (now-idle) load ring: they fire the
        # moment the last load completes, covering the final macros'
        # recurrence/blend chains with pure DMA drain.
        for e in E_SET:
            for g in range(NG):
                nc.sync.dma_start(out=z_out_ap(e, g), in_=xgE[e][g])

    if not nc.is_finalized():
        nc.finalize()
    return nc


def _get_program(rows, num_steps, neg_inv, exp_bias, neg_h):
    key = (rows, num_steps, neg_inv, exp_bias, neg_h)
    if key not in _PROGRAM_CACHE:
        _PROGRAM_CACHE[key] = _build_program(rows, num_steps, neg_inv,
                                             exp_bias, neg_h)
    return _PROGRAM_CACHE[key]


def _bf16_rtn(x32: np.ndarray) -> np.ndarray:
    """fp32 -> bf16 with round-to-nearest-even (no inf/nan handling)."""
    u = x32.view(np.uint32)
    r = (u + np.uint32(0x7FFF) + ((u >> np.uint32(16)) & np.uint32(1)))
    return (r >> np.uint32(16)).astype(np.uint16).view(ml_dtypes.bfloat16)


def _pack_core(xb_core: np.ndarray, sizes) -> np.ndarray:
    """[rows, D] bf16 -> flat packed (m, g, p, j, r) order."""
    parts = []
    r0 = 0
    for Rm in sizes:
        xv = xb_core[r0 : r0 + Rm].reshape(Rm, NG, JPG, 128)
        parts.append(np.ascontiguousarray(xv.transpose(1, 3, 2, 0)).reshape(-1))
        r0 += Rm
    return np.concatenate(parts)


def kernel(x, manifold_mu, manifold_U, manifold_S, attractor_mu,
           log_step, sigma, num_steps):
    global LAST_RESULT
    x = np.ascontiguousarray(np.asarray(x, dtype=np.float32))
    mu = np.asarray(manifold_mu, dtype=np.float64)
    U = np.asarray(manifold_U, dtype=np.float64)
    S = np.asarray(manifold_S, dtype=np.float64)
    tgt = np.asarray(attractor_mu, dtype=np.float64)
    ls = float(np.asarray(log_step))
    sg = float(np.asarray(sigma))
    ns = int(np.asarray(num_steps))

    batch, dmodel = x.shape
    assert dmodel == D and batch % N_CORES == 0

    if ns <= 0:
        return x.copy()

    # Host-side parameter folding (O(D*K), trivial). qT/wt/C use the
    # bf16-rounded W so they are consistent with the device projection.
    W = U / (S + 1e-6)[None, :]
    W16 = W.astype(ml_dtypes.bfloat16)
    Wq = W16.astype(np.float64)
    qT = tgt @ Wq
    qmu = mu @ Wq
    wt = qT - qmu
    Cc = float(wt @ wt)
    inv = 1.0 / (float(KSUB) * 2.0 * sg * sg * 1.0)  # TEMPERATURE = 1.0
    step = min(max(math.exp(ls), 1e-3), 1.0)
    h = step / ns

    neg_inv = -inv
    exp_bias = -inv * Cc
    neg_h = -h

    rows = batch // N_CORES
    nc = _get_program(rows, ns, neg_inv, exp_bias, neg_h)
    sizes = _schedule(rows)

    abr = np.zeros((128, 2), ml_dtypes.bfloat16)
    abr[0:KSUB, 0] = 1.0
    abr[KSUB:128, 1] = (2.0 * wt).astype(ml_dtypes.bfloat16)
    common = {
        "w": np.ascontiguousarray(W16),
        "nqt": np.ascontiguousarray((-qT).astype(np.float32)[:, None]),
        "abr": abr,
    }

    xb = _bf16_rtn(x)
    in_maps = []
    for i in range(N_CORES):
        xpk = _pack_core(xb[i * rows : (i + 1) * rows], sizes)
        in_maps.append({"x": xpk, **common})

    trace = bool(int(os.environ.get("GOF_TRACE", "0")))
    res = run_bass_kernel_spmd(nc, in_maps, list(range(N_CORES)), trace=trace)
    LAST_RESULT = res

    t32 = tgt.astype(np.float32)
    out = np.empty((batch, D), np.float32)
    for i in range(N_CORES):
        cc = np.asarray(res.results[i]["c"], np.float32)
        c_core = np.ascontiguousarray(cc.T).reshape(-1)
        zu = np.asarray(res.results[i]["z"]).view(np.uint16)
        oc = out[i * rows : (i + 1) * rows]
        r0 = 0
        off = 0
        for Rm in sizes:
            n = 128 * NJ * Rm
            zv = zu[off : off + n].reshape(NG, 128, JPG, Rm) \
                .transpose(3, 0, 2, 1)  # (r, g, j, p)
            np.left_shift(zv.astype(np.uint32).reshape(Rm, D), 16,
                          out=oc[r0 : r0 + Rm].view(np.uint32))
            r0 += Rm
            off += n
        oc += np.outer(1.0 - c_core, t32)
    return out
